# revision 2
# baseline (speedup 1.0000x reference)
"""Trainium2 Bass kernel for nn_AgentEmbedding (embedding_lookup).

Contract: kernel(**inputs) takes the FULL unsharded inputs (numpy arrays,
keyed as in setup_inputs()) and returns the FULL [64, 50, 128] float32
output. Internally the batch dim B=64 is sharded 8-ways (8 per core);
the small linear weights are algebraically fused on the host (the module
is linear end-to-end) and replicated.

Per-core device program (B_local=8, T=400 tokens, E=128):
  1. 7 indirect-DMA gathers (one row id per dest partition, chunk-paired
     order so each 128-token chunk's two tiles arrive back to back) pull
     the 2*T=800 rows from the flat [80000, 128] table.  The table is
     pre-cast to bf16 on the host (256B rows; src==dst dtype, so no
     SWDGE cast penalty) - halves the gather transfer bytes.
     The ~1.1us/instr Q7 descriptor generation is the serial bottleneck;
     batched multi-row offset grids (broken lowering), dma_gather
     (9us Q7 LIBRARY_RELOAD + 11.5ns/idx), and DRAM-resident offset
     tables (walrus: must be SBUF) were all measured/rejected.
  2. Each gathered tile is PE-transposed to feature-major (bf16 identity
     shipped in wmain); the mandatory PSUM->SBUF copies alternate
     between DVE and ACT so the tail chain never serializes on one
     engine.
  3. All accumulation is token-major bf16 in PSUM (bf16 matmuls run at
     1 cyc/row vs 2-4 for narrow fp32r): per chunk a [20-partition]
     stacked matmul (one-hot batch selector -> on-device graph
     projection, features, ones -> fused bias) opens the PSUM group
     EARLY (before the gathers land), then the two gathered-embedding
     matmuls close it.  One copy PSUM->SBUF (DVE/ACT alternating), one
     DMA out per chunk (sync/scalar alternating).
"""

import os
import numpy as np

B, M, N, E = 64, 50, 10000, 128
NCORES = 8
BL = B // NCORES            # batches per core
T = BL * M                  # tokens per core
NG = 7                      # gather instructions
CHUNKS = [(0, 128), (128, 128), (256, 128), (384, 16)]  # output chunks
PRECAST_BF16 = True         # host pre-casts the table to bf16 (no DMA cast)
CAST_IN_DMA = False         # gather casts fp32->bf16 inside the DMA
NWARM = 10

_cache = {}

last_exec_time_ns = None


def _install_trace_shims():
    """antenv.axon_hooks is absent in this image; register the NTFF hook
    ourselves so run_bass_kernel_spmd(trace=True) works under axon."""
    import sys, types
    if "antenv.axon_hooks" not in sys.modules:
        mod = types.ModuleType("antenv.axon_hooks")
        store = {}
        mod.set_axon_ntff_profile_hook = lambda h: store.__setitem__("h", h)
        mod.get_axon_ntff_profile_hook = lambda: store.get("h")
        sys.modules["antenv.axon_hooks"] = mod
        try:
            from trn_agent_boot.trn_boot import _ntff_profile_via_ctypes
            mod.set_axon_ntff_profile_hook(
                _ntff_profile_via_ctypes("/opt/axon/libaxon_pjrt.so")
            )
        except Exception:
            pass
    import concourse.bass_utils as bu
    bu.upload_artifacts = lambda d: d  # zero-egress container


def _grid():
    """Gather slot (p, j) -> (k, token, valid). Col 2c holds chunk c's k=0
    tokens, col 2c+1 its k=1 tokens (c<3); col 6 packs both 16-token
    chunk-3 tails."""
    k_grid = np.zeros((128, NG), np.int64)
    t_grid = np.zeros((128, NG), np.int64)
    valid = np.zeros((128, NG), bool)
    for j in range(6):
        c, k = j // 2, j % 2
        k_grid[:, j] = k
        t_grid[:, j] = c * 128 + np.arange(128)
        valid[:, j] = True
    k_grid[0:16, 6], t_grid[0:16, 6], valid[0:16, 6] = 0, 384 + np.arange(16), True
    k_grid[16:32, 6], t_grid[16:32, 6], valid[16:32, 6] = 1, 384 + np.arange(16), True
    return k_grid, t_grid, valid


def _build_nc():
    """Build + compile the per-core Bass program (SPMD: same program on
    all 8 cores, per-core input data)."""
    import concourse.bass as bass
    import concourse.bacc as bacc
    import concourse.mybir as mybir
    import concourse.tile as tile
    from concourse.masks import make_identity

    f32 = mybir.dt.float32
    bf16 = mybir.dt.bfloat16
    i32 = mybir.dt.int32
    gdt = bf16 if (CAST_IN_DMA or PRECAST_BF16) else f32
    WCOLS = 520 if (CAST_IN_DMA or PRECAST_BF16) else 392
    Copy = mybir.ActivationFunctionType.Copy

    nc = bacc.Bacc("TRN2", target_bir_lowering=False,
                   dynamic_dma_scratch_size=65536,
                   disable_frame_to_traceback=True)
    with tile.TileContext(nc) as tc:
        with tc.tile_pool(name="dram", bufs=1, space="DRAM") as dram:
            cities = dram.tile([BL * N, E], gdt, kind="ExternalInput", name="cities")
            idx2 = dram.tile([128, 8], i32, kind="ExternalInput", name="idx2")
            featX = dram.tile([20, 528], bf16, kind="ExternalInput", name="featX")
            wmain = dram.tile([128, WCOLS], bf16, kind="ExternalInput", name="wmain")
            out = dram.tile([T, E], f32, kind="ExternalOutput", name="out")
            names = dict(cities=cities.name, idx2=idx2.name, featX=featX.name,
                         wmain=wmain.name, out=out.name)

            with (
                tc.tile_pool(name="sb", bufs=1) as sb,
                tc.tile_pool(name="sbo", bufs=4) as sbo,
                tc.tile_pool(name="psT", bufs=3, space="PSUM") as psT,
                tc.tile_pool(name="psG", bufs=1, space="PSUM") as psG,
                tc.tile_pool(name="psD", bufs=4, space="PSUM") as psD,
            ):
                idxi = sb.tile([128, 8], i32, name="idxi")
                nc.sync.dma_start(out=idxi[:], in_=idx2[:])
                wmain_sb = sb.tile([128, WCOLS], bf16, name="wmain_sb")
                nc.sync.dma_start(out=wmain_sb[:], in_=wmain[:])
                featX_sb = sb.tile([20, 528], bf16, name="featX_sb")
                nc.scalar.dma_start(out=featX_sb[:], in_=featX[:])

                if gdt == f32:
                    ident = sb.tile([128, 128], f32, name="ident")
                    make_identity(nc, ident[:])

                # graph projection Gg[b, o] -> bf16 rows 0-7 of the stacked rhs
                psg = psG.tile([8, 128], f32, name="psg")
                nc.tensor.matmul(out=psg[:, :], lhsT=wmain_sb[:, 384:392],
                                 rhs=wmain_sb[:, 0:128], start=True, stop=True)
                nc.vector.tensor_copy(out=featX_sb[0:8, 400:528], in_=psg[:, :])

                # open each chunk's PSUM group with the stacked feat/graph/bias
                # matmul -- no gather dependency, runs during the gathers
                pgs = []
                for c, (o, cnt) in enumerate(CHUNKS):
                    pg = psD.tile([128, 128], f32, tag="pg", name=f"pg_{c}")
                    nc.tensor.matmul(out=pg[:cnt, :],
                                     lhsT=featX_sb[:, o:o + cnt],
                                     rhs=featX_sb[:, 400:528],
                                     start=True, stop=False)
                    pgs.append(pg)

                # keep the PE busy through the gather window so the HAM
                # clock-gate is released (2.4 GHz) before the chunk matmuls
                for w in range(NWARM):
                    nc.tensor.matmul(out=psg[:, :], lhsT=wmain_sb[:, 384:392],
                                     rhs=wmain_sb[:, 0:128],
                                     start=True, stop=True)

                # 7 gathers in chunk-pair order; transpose + copy to bf16
                gT = {}
                for j in range(NG):
                    cnt = 32 if j == 6 else 128
                    ga = sb.tile([128, E], gdt, name=f"ga_{j}")
                    nc.gpsimd.indirect_dma_start(
                        out=ga[:cnt, :],
                        out_offset=None,
                        in_=cities[:, :],
                        in_offset=bass.IndirectOffsetOnAxis(
                            ap=idxi[:cnt, j:j + 1], axis=0),
                    )
                    pt = psT.tile([128, 128], gdt, tag="pt", name=f"pt_{j}")
                    if gdt == bf16:
                        ident_ap = wmain_sb[:cnt, 392:392 + cnt]
                    else:
                        ident_ap = ident[:cnt, :cnt]
                    nc.tensor.transpose(out=pt[:, :cnt], in_=ga[:cnt, :],
                                        identity=ident_ap)
                    g = sb.tile([128, 128], bf16, name=f"gT_{j}")
                    if j % 2 == 0:
                        nc.vector.tensor_copy(out=g[:, :cnt], in_=pt[:, :cnt])
                    else:
                        nc.scalar.activation(out=g[:, :cnt], in_=pt[:, :cnt],
                                             func=Copy)
                    gT[j] = g
                    # close chunk c's PSUM group once its pair is ready
                    if j in (1, 3, 5, 6):
                        c = j // 2
                        o, cnt_c = CHUNKS[c]
                        if c == 3:
                            l0, l1 = gT[6][:, 0:16], gT[6][:, 16:32]
                        else:
                            l0, l1 = gT[2 * c][:, :], gT[2 * c + 1][:, :]
                        pg = pgs[c]
                        nc.tensor.matmul(out=pg[:cnt_c, :], lhsT=l0,
                                         rhs=wmain_sb[:, 128:256],
                                         start=False, stop=False)
                        nc.tensor.matmul(out=pg[:cnt_c, :], lhsT=l1,
                                         rhs=wmain_sb[:, 256:384],
                                         start=False, stop=True)
                        ob = sbo.tile([128, E], f32, tag="ob", name=f"ob_{c}")
                        if c % 2 == 0:
                            nc.vector.tensor_copy(out=ob[:cnt_c, :],
                                                  in_=pg[:cnt_c, :])
                        else:
                            nc.scalar.activation(out=ob[:cnt_c, :],
                                                 in_=pg[:cnt_c, :], func=Copy)
                        eng = nc.sync if c % 2 == 0 else nc.scalar
                        eng.dma_start(out=out[o:o + cnt_c, :], in_=ob[:cnt_c, :])

    nc.compile()
    return nc, names


def _host_prep(inputs):
    """Fuse the linear layers (the module has no nonlinearity) and lay out
    per-core device inputs."""
    import ml_dtypes
    bf = ml_dtypes.bfloat16
    f64 = np.float64
    W_a = np.asarray(inputs["W_a"], f64)
    Wa0, Wa1 = W_a[:, :E], W_a[:, E:]
    W_dp = np.asarray(inputs["W_dp"], f64)
    Wf0 = Wa1 @ W_dp[:, :E]
    Wf1 = Wa1 @ W_dp[:, E:]
    Wfc = Wa1 @ np.asarray(inputs["W_dc"], f64)
    Wfn = Wa1 @ np.asarray(inputs["W_nc"], f64)
    Wfp = Wa1 @ np.asarray(inputs["W_ps"], f64)
    Wfg = Wa0 @ np.asarray(inputs["W_g"], f64)
    b_sum = (np.asarray(inputs["b_dp"], f64) + np.asarray(inputs["b_dc"], f64)
             + np.asarray(inputs["b_nc"], f64) + np.asarray(inputs["b_ps"], f64))
    b_total = (np.asarray(inputs["b_a"], f64) + Wa1 @ b_sum
               + Wa0 @ np.asarray(inputs["b_g"], f64))
    Wff = np.concatenate([Wfc, Wfn, Wfp], axis=1)  # [128, 11]

    WCOLS = 520 if (CAST_IN_DMA or PRECAST_BF16) else 392
    # wmain: [WfgT | Wf0T | Wf1T | graphT(per-core) | identity(bf16 mode)]
    wmain_base = np.zeros((128, WCOLS), np.float32)
    wmain_base[:, 0:128] = Wfg.T
    wmain_base[:, 128:256] = Wf0.T
    wmain_base[:, 256:384] = Wf1.T
    if CAST_IN_DMA or PRECAST_BF16:
        wmain_base[:, 392:520] = np.eye(128, dtype=np.float32)

    sel = np.zeros((8, T), np.float32)
    sel[np.arange(T) // M, np.arange(T)] = 1.0

    cities_embed = np.asarray(inputs["cities_embed"], np.float32)
    graph_embed = np.asarray(inputs["graph_embed"], np.float32)
    agent_state = np.asarray(inputs["agent_state"], np.float32)

    k_grid, t_grid, valid = _grid()
    bofs = (t_grid // M) * N * valid

    in_maps = []
    for core in range(NCORES):
        bsl = slice(core * BL, (core + 1) * BL)
        ag = agent_state[bsl].reshape(T, 13)
        idx2 = np.zeros((128, 8), np.int32)
        idx2[:, 0:NG] = (ag[t_grid, k_grid].astype(np.int64) * valid
                         + bofs).astype(np.int32)
        featX = np.zeros((20, 528), np.float32)
        featX[0:8, :T] = sel
        featX[8:19, :T] = ag[:, 2:13].T
        featX[19, :T] = 1.0
        featX[8:19, 400:528] = Wff.T
        featX[19, 400:528] = b_total
        wmain = wmain_base.copy()
        wmain[:, 384:392] = graph_embed[bsl, 0, :].T
        in_maps.append({
            "cities": np.ascontiguousarray(
                (cities_embed[bsl].astype(bf) if PRECAST_BF16
                 else cities_embed[bsl]).reshape(BL * N, E)),
            "idx2": idx2,
            "featX": featX.astype(bf),
            "wmain": wmain.astype(bf),
        })
    return in_maps


def kernel(**inputs):
    global last_exec_time_ns
    trace = os.environ.get("BASS_KERNEL_TRACE", "") == "1"
    if trace:
        _install_trace_shims()

    from concourse.bass_utils import run_bass_kernel_spmd

    if "nc" not in _cache:
        _cache["nc"], _cache["names"] = _build_nc()
    nc, names = _cache["nc"], _cache["names"]

    in_maps = []
    for m in _host_prep(inputs):
        in_maps.append({names[k]: v for k, v in m.items()})

    kwargs = {}
    if trace:
        tdir = os.environ.get("BASS_KERNEL_TRACE_DIR", "/tmp/kern_trace")
        import shutil
        shutil.rmtree(tdir, ignore_errors=True)
        os.makedirs(tdir, exist_ok=True)
        kwargs = dict(trace=True, trace_cores=list(range(NCORES)), tmpdir=tdir)
    res = run_bass_kernel_spmd(nc, in_maps, core_ids=list(range(NCORES)), **kwargs)
    last_exec_time_ns = res.exec_time_ns

    out = np.stack([res.results[i][names["out"]] for i in range(NCORES)])
    return out.reshape(B, M, E).astype(np.float32)

